# revision 36
# baseline (speedup 1.0000x reference)
"""Trainium2 Bass kernel for nn_FOGCNConv (GNN message passing).

Math (reference):
    weight = softmax(importance, axis=0)            # [C, F]
    edge_score = cnt @ weight                       # [E, F]
    msgs = embedding[src] * edge_score              # [E, F]
    new_embedding = segment_sum(msgs, dst, N)       # [N, F]
    node_score = segment_sum(edge_score, dst, N)    # [N, F]
    out = new_embedding / node_score

Key structural facts (hardcoded; guaranteed by the input spec):
    N=20000 nodes, E=640000 edges, C=64, F=128, and dst is a permutation of
    arange(E) % N  =>  every node has exactly DEG=32 incoming edges.

Strategy:
  - Host: sort edges by dst. Then the two segment-sums become perfectly
    regular reductions over groups of 32 consecutive edges.
  - Shard *contiguous dst ranges* across the 8 cores (2500 nodes / 80000
    edges each) => fully independent cores, no collectives.
  - Device (per core, per 128-node window = 4096 edges = 32 edge tiles):
      * dma_gather embedding rows for the window's src indices
        (edge e -> partition e%128, free block e//128), 1024 idxs per
        gather (SWDGE ring capacity), rotated over 4 SWDGE queues.
      * edge_score for tile pair (j, j+16) in ONE K=128 fp16 matmul:
        lhsT = packed cnt^T [128, 128] (c-halves stacked), rhs = Wstack
        [128, 256] block-diagonal softmax weights -> es [128e, 256] f32.
      * msgs = gathered * edge_score on DVE (one 3D-strided op per pair),
        cast to fp16.
      * new_embedding window [128f, 128n] via PE "segment matmuls": per
        edge tile t, out[:, 4t:4t+4] = msgs_t^T @ P where P[e, j] =
        (e//32 == j) is a constant one-hot (edges are dst-sorted).
      * node_score factored: segsum(cnt) via DVE strided reduce ->
        block-diagonal fp16 redx, then ONE matmul ns^T = w2^T @ redx.
      * out window = new_embedding * 1/node_score (DVE), F-major.
  - cnt is fed pre-transposed fp16 and "half-packed" to [128, ...] so DMA
    uses all 128 partitions: partition (half*64 + c) holds cnt^T[c] for
    the window's half-th group of 2048 edges.
  - PE-row-position rule: a PSUM bank must only ever be written by matmuls
    whose stationary operand sits at one SBUF base partition. All matmuls
    here use base partition 0. (Mixing positions in one bank crashes the
    device; so does gpsimd.partition_all_reduce, and so does a dma_gather
    of more than ring-capacity indices.)
"""

import sys

if "/opt/trn_rl_repo" not in sys.path:
    sys.path.insert(0, "/opt/trn_rl_repo")

import numpy as np

# Problem sizes (fixed by the spec).
N_NODES = 20000
N_EDGES = 640000
C = 64
F = 128
N_CORES = 8
NPC = N_NODES // N_CORES       # 2500 nodes per core
EPC = N_EDGES // N_CORES       # 80000 edges per core
DEG = N_EDGES // N_NODES       # 32 edges per node
WIN_NODES = 128                # nodes per window
EPW = WIN_NODES * DEG          # 4096 edges per window
N_WIN = -(-NPC // WIN_NODES)   # 20 windows per core (last partial: 68 nodes)
PAD_EPC = N_WIN * EPW          # 81920 padded edges per core
HALF = EPW // 2                # 2048
GATHER_CHUNK = 1024            # max idxs per dma_gather (SWDGE ring capacity)
N_QUEUES = 4                   # SWDGE queues; rotate gathers across them

_CACHE = {}


def _build_nc(n_win=N_WIN, skip=()):
    import concourse.bass as bass  # noqa: F401
    import concourse.bacc as bacc
    import concourse.tile as tile
    import concourse.mybir as mybir
    from contextlib import ExitStack

    f32 = mybir.dt.float32
    f16 = mybir.dt.float16
    i16 = mybir.dt.int16
    AF = mybir.ActivationFunctionType
    AX = mybir.AxisListType

    nc = bacc.Bacc("TRN2", target_bir_lowering=False, debug=False,
                   num_swdge_queues=N_QUEUES)
    cntp = nc.declare_dram_parameter("cntp", [128, N_WIN * HALF], f16, isOutput=False)
    idx = nc.declare_dram_parameter("idx", [128, PAD_EPC // 16], i16, isOutput=False)
    emb = nc.declare_dram_parameter("emb", [N_NODES, F], f32, isOutput=False)
    imp = nc.declare_dram_parameter("imp", [C, F], f32, isOutput=False)
    pbase = nc.declare_dram_parameter("pbase", [128, 4], f16, isOutput=False)
    out = nc.declare_dram_parameter("out", [F, NPC], f32, isOutput=True)

    with ExitStack() as ctx:
        tc = ctx.enter_context(tile.TileContext(nc))
        const = ctx.enter_context(tc.tile_pool(name="const", bufs=1))

        # ---- constants ----
        pbase_sb = const.tile([128, 4], f16)
        nc.sync.dma_start(pbase_sb[:], pbase[:, :])

        # ---- softmax(importance, axis=0) on DVE (block transposes) ----
        imp_sb = const.tile([C, F], f32)
        nc.sync.dma_start(imp_sb[:], imp[:, :])
        exp_sb = const.tile([C, F], f32)
        nc.scalar.activation(exp_sb[:], imp_sb[:], AF.Exp)
        expT_sb = const.tile([128, C], f32)
        for i in range(C // 32):
            for j in range(F // 32):
                nc.vector.transpose(
                    expT_sb[32 * j:32 * j + 32, 32 * i:32 * i + 32],
                    exp_sb[32 * i:32 * i + 32, 32 * j:32 * j + 32])
        s_sb = const.tile([128, 1], f32)
        nc.vector.reduce_sum(s_sb[:], expT_sb[:], axis=AX.X)
        rec_sb = const.tile([128, 1], f32)
        nc.vector.reciprocal(rec_sb[:], s_sb[:])
        wT_sb = const.tile([128, C], f32)
        nc.vector.tensor_scalar_mul(wT_sb[:], expT_sb[:], rec_sb[:])
        w2_sb = const.tile([128, F], f32)
        for i in range(F // 32):
            for j in range(C // 32):
                nc.vector.transpose(
                    w2_sb[32 * j:32 * j + 32, 32 * i:32 * i + 32],
                    wT_sb[32 * i:32 * i + 32, 32 * j:32 * j + 32])
        nc.scalar.dma_start(w2_sb[C:128, :], w2_sb[0:C, :])

        # fp16 block-diagonal Wstack for the paired edge-score matmuls.
        wstack = const.tile([128, 2 * F], f16)
        nc.vector.memset(wstack[:], 0.0)
        nc.vector.tensor_copy(wstack[0:C, 0:F], w2_sb[0:C, :])
        nc.vector.tensor_copy(wstack[C:128, F:2 * F], w2_sb[C:128, :])

        out_sb = const.tile([128, NPC], f32)

        cnt_pool = ctx.enter_context(tc.tile_pool(name="cnt", bufs=2))
        idx_pool = ctx.enter_context(tc.tile_pool(name="idx", bufs=3))
        gath_pool = ctx.enter_context(tc.tile_pool(name="gath", bufs=4))
        es_pool = ctx.enter_context(tc.tile_pool(name="es", bufs=3, space="PSUM"))
        msgs_pool = ctx.enter_context(tc.tile_pool(name="msgs", bufs=3))
        ne_pool = ctx.enter_context(tc.tile_pool(name="ne", bufs=2, space="PSUM"))
        ns_pool = ctx.enter_context(tc.tile_pool(name="ns", bufs=1, space="PSUM"))
        red_pool = ctx.enter_context(tc.tile_pool(name="red", bufs=2))
        rtree_pool = ctx.enter_context(tc.tile_pool(name="rtree", bufs=2))
        rns_pool = ctx.enter_context(tc.tile_pool(name="rns", bufs=2))

        gq = 0  # rotating SWDGE queue index

        # ---- main loop over 128-node windows ----
        # Edges are padded to whole windows with cnt=0 / idx=0, so every
        # window runs the full 32 tiles; only the final column copies are
        # restricted to the window's real node count.
        for w in range(n_win):
            nodes_w = min(WIN_NODES, NPC - w * WIN_NODES)
            nt = (nodes_w * DEG) // 128      # real edge tiles (32; last: 17)
            n_idx = nt * 128

            cnt_sb = cnt_pool.tile([128, HALF], f16, tag="cnt")
            nc.sync.dma_start(cnt_sb[:], cntp[:, w * HALF:(w + 1) * HALF])

            idx_sb = idx_pool.tile([128, EPW // 16], i16, tag="idx")
            nc.sync.dma_start(
                idx_sb[:, :n_idx // 16],
                idx[:, w * (EPW // 16): w * (EPW // 16) + n_idx // 16])

            gath = gath_pool.tile([128, EPW], f32, tag="gath")
            if "gather" in skip:
                nc.vector.memset(gath[:], 1.0)
            else:
                gath3 = gath[:].rearrange("p (t f) -> p t f", f=F)
                for e0 in range(0, n_idx, GATHER_CHUNK):
                    ecnt = min(GATHER_CHUNK, n_idx - e0)
                    nc.gpsimd.dma_gather(
                        out_ap=gath3[:, e0 // 128:(e0 + ecnt) // 128, :],
                        in_ap=emb[:, :],
                        idxs_ap=idx_sb[:, e0 // 16:(e0 + ecnt) // 16],
                        num_idxs=ecnt,
                        num_idxs_reg=ecnt,
                        elem_size=F,
                        queue_num=gq,
                    )
                    gq = (gq + 1) % N_QUEUES

            ne_ps = ne_pool.tile([128, 128], f32, tag="ne")
            g3 = gath[:].rearrange("p (t f) -> p t f", f=F)
            if nt == 32:
                # two tile pairs (j, j+16), (j+1, j+17) per PSUM bank; one
                # [128, 512] DVE multiply for all four tiles.
                for j in range(0, 16, 2):
                    es_ps = es_pool.tile([128, 512], f32, tag="es")
                    nc.tensor.matmul(
                        es_ps[:, 0:2 * F], cnt_sb[:, 128 * j:128 * (j + 1)],
                        wstack[:], start=True, stop=True)
                    nc.tensor.matmul(
                        es_ps[:, 2 * F:4 * F],
                        cnt_sb[:, 128 * (j + 1):128 * (j + 2)],
                        wstack[:], start=True, stop=True)
                    msgs = msgs_pool.tile([128, 512], f16, tag="msgs")
                    g4 = gath[:].rearrange(
                        "p (h j f) -> p j h f", h=2, f=F)  # t = h*16 + j
                    nc.vector.tensor_mul(
                        msgs[:].rearrange("p (j h f) -> p j h f", j=2, f=F),
                        g4[:, j:j + 2, :, :],
                        es_ps[:].rearrange("p (j h f) -> p j h f", j=2, f=F),
                    )
                    for a, t in enumerate((j, j + 16, j + 1, j + 17)):
                        nc.tensor.matmul(
                            ne_ps[:, 4 * t:4 * t + 4],
                            msgs[:, a * F:(a + 1) * F], pbase_sb[:],
                            start=True, stop=True,
                        )
            else:
                for j in range(min(nt, 16)):
                    has_hi = j + 16 < nt
                    es_ps = es_pool.tile([128, 512], f32, tag="es")
                    nw = 2 * F if has_hi else F
                    nc.tensor.matmul(
                        es_ps[:, :nw], cnt_sb[:, 128 * j:128 * (j + 1)],
                        wstack[:, :nw],
                        start=True, stop=True,
                    )
                    msgs = msgs_pool.tile([128, 512], f16, tag="msgs")
                    if has_hi:
                        nc.vector.tensor_mul(
                            msgs[:, :2 * F].rearrange("p (t f) -> p t f", f=F),
                            g3[:, j:j + 17:16, :],
                            es_ps[:, :2 * F].rearrange("p (t f) -> p t f", f=F),
                        )
                    else:
                        nc.vector.tensor_mul(
                            msgs[:, 0:F], g3[:, j, :], es_ps[:, 0:F])
                    nc.tensor.matmul(
                        ne_ps[:, 4 * j:4 * j + 4],
                        msgs[:, 0:F], pbase_sb[:],
                        start=True, stop=True,
                    )
                    if has_hi:
                        nc.tensor.matmul(
                            ne_ps[:, 64 + 4 * j:64 + 4 * j + 4],
                            msgs[:, F:2 * F], pbase_sb[:],
                            start=True, stop=True,
                        )

            # node_score path: segsum(cnt) on DVE, then ns^T = W^T @ segsum.
            # Two f32 matmuls in position-dedicated PSUM banks (lo: PE rows
            # 0-63, hi: rows 64-127) to honor the PE-row-position rule.
            # Pairwise add tree on GpSimd: its only other work is gather
            # descriptor generation, and its ring-stall slack absorbs this,
            # taking ~2.3us/window off the critical DVE engine.
            red_sb = red_pool.tile([128, 64], f32, tag="red")
            cnt3 = cnt_sb[:].rearrange("p (g d) -> p g d", d=DEG)
            t1 = rtree_pool.tile([128, 1024], f32, tag="t1")
            t1v = t1[:].rearrange("p (g d) -> p g d", d=16)
            nc.gpsimd.tensor_add(t1v, cnt3[:, :, 0:16], cnt3[:, :, 16:32])
            t2 = rtree_pool.tile([128, 512], f32, tag="t2")
            t2v = t2[:].rearrange("p (g d) -> p g d", d=8)
            nc.gpsimd.tensor_add(t2v, t1v[:, :, 0:8], t1v[:, :, 8:16])
            t3 = rtree_pool.tile([128, 256], f32, tag="t3")
            t3v = t3[:].rearrange("p (g d) -> p g d", d=4)
            nc.gpsimd.tensor_add(t3v, t2v[:, :, 0:4], t2v[:, :, 4:8])
            t4 = rtree_pool.tile([128, 128], f32, tag="t4")
            t4v = t4[:].rearrange("p (g d) -> p g d", d=2)
            nc.gpsimd.tensor_add(t4v, t3v[:, :, 0:2], t3v[:, :, 2:4])
            nc.gpsimd.tensor_add(
                red_sb[:].rearrange("p (g d) -> p g d", d=1),
                t4v[:, :, 0:1], t4v[:, :, 1:2])
            lo_w = min(64, nodes_w)
            hi_w = nodes_w - lo_w
            ns_lo = ns_pool.tile([128, 64], f32, tag="ns_lo")
            nc.tensor.matmul(ns_lo[:], w2_sb[0:64, :], red_sb[0:64, :],
                             start=True, stop=True)
            rns_sb = rns_pool.tile([128, 128], f32, tag="rns")
            nc.vector.reciprocal_approx_fast(rns_sb[:, :lo_w], ns_lo[:, :lo_w])
            if hi_w > 0:
                ns_hi = ns_pool.tile([128, 64], f32, tag="ns_hi")
                nc.tensor.matmul(ns_hi[:], w2_sb[64:128, :], red_sb[64:128, :],
                                 start=True, stop=True)
                nc.vector.reciprocal_approx_fast(rns_sb[:, 64:64 + hi_w], ns_hi[:, :hi_w])
            nc.vector.tensor_mul(
                out_sb[:, w * WIN_NODES:w * WIN_NODES + nodes_w],
                ne_ps[:, :nodes_w],
                rns_sb[:, :nodes_w],
            )

        nc.sync.dma_start(out[:, :], out_sb[:, :NPC])

    nc.compile()
    return nc


def get_nc():
    if "nc" not in _CACHE:
        _CACHE["nc"] = _build_nc()
    return _CACHE["nc"]


def prep_in_maps(inputs):
    cnt = np.asarray(inputs["cnt"], dtype=np.float32)
    emb = np.ascontiguousarray(np.asarray(inputs["embedding"], dtype=np.float32))
    imp = np.ascontiguousarray(np.asarray(inputs["importance"], dtype=np.float32))
    src = np.asarray(inputs["src"], dtype=np.int64)
    dst = np.asarray(inputs["dst"], dtype=np.int64)

    perm = np.argsort(dst, kind="stable")
    src_s = src[perm]
    cnt_s = cnt[perm].astype(np.float16)

    pbase = np.zeros((128, 4), np.float16)
    pbase[np.arange(128), np.arange(128) // DEG] = 1.0

    in_maps = []
    for c in range(N_CORES):
        sl = slice(c * EPC, (c + 1) * EPC)
        cnt_core = np.zeros((PAD_EPC, C), np.float16)
        cnt_core[:EPC] = cnt_s[sl]
        src_core = np.zeros((PAD_EPC,), np.int64)
        src_core[:EPC] = src_s[sl]
        # half-pack: [w, half, j, c] -> [half*64+c, w*HALF+j]
        cc = cnt_core.reshape(N_WIN, 2, HALF, C)
        cntp = np.ascontiguousarray(
            cc.transpose(1, 3, 0, 2).reshape(128, N_WIN * HALF))
        # wrapped int16 index layout: idx i at [i%16, i//16], replicated x8
        idxw = np.ascontiguousarray(
            np.tile(src_core.reshape(PAD_EPC // 16, 16).T, (8, 1)).astype(np.int16))
        in_maps.append({
            "cntp": cntp,
            "idx": idxw,
            "emb": emb,
            "imp": imp,
            "pbase": pbase,
        })
    return in_maps


def unshard(core_outs):
    # each core out: [F, NPC] (F-major); concat over node axis, transpose.
    full = np.concatenate(core_outs, axis=1)          # [F, N]
    return np.ascontiguousarray(full.T.astype(np.float32))


def run(inputs, trace=False):
    from concourse.bass_utils import run_bass_kernel_spmd

    nc = get_nc()
    in_maps = prep_in_maps(inputs)
    res = run_bass_kernel_spmd(
        nc, in_maps, core_ids=list(range(N_CORES)), trace=trace)
    outs = [res.results[i]["out"] for i in range(N_CORES)]
    return unshard(outs), res


def kernel(**inputs):
    out, _ = run(inputs, trace=False)
    return out


# revision 37
# speedup vs baseline: 6.6608x; 6.6608x over previous
"""Trainium2 Bass kernel for nn_FOGCNConv (GNN message passing).

Math (reference):
    weight = softmax(importance, axis=0)            # [C, F]
    edge_score = cnt @ weight                       # [E, F]
    msgs = embedding[src] * edge_score              # [E, F]
    new_embedding = segment_sum(msgs, dst, N)       # [N, F]
    node_score = segment_sum(edge_score, dst, N)    # [N, F]
    out = new_embedding / node_score

Key structural facts (hardcoded; guaranteed by the input spec):
    N=20000 nodes, E=640000 edges, C=64, F=128, and dst is a permutation of
    arange(E) % N  =>  every node has exactly DEG=32 incoming edges.

Strategy:
  - Host: sort edges by dst. Then the two segment-sums become perfectly
    regular reductions over groups of 32 consecutive edges.
  - Shard *contiguous dst ranges* across the 8 cores (2500 nodes / 80000
    edges each) => fully independent cores, no collectives.
  - Device (per core, per 128-node window = 4096 edges = 32 edge tiles):
      * dma_gather embedding rows for the window's src indices
        (edge e -> partition e%128, free block e//128), 1024 idxs per
        gather (SWDGE ring capacity), rotated over 4 SWDGE queues.
      * edge_score for tile pair (j, j+16) in ONE K=128 fp16 matmul:
        lhsT = packed cnt^T [128, 128] (c-halves stacked), rhs = Wstack
        [128, 256] block-diagonal softmax weights -> es [128e, 256] f32.
      * msgs = gathered * edge_score on DVE (one 3D-strided op per pair),
        cast to fp16.
      * new_embedding window [128f, 128n] via PE "segment matmuls": per
        edge tile t, out[:, 4t:4t+4] = msgs_t^T @ P where P[e, j] =
        (e//32 == j) is a constant one-hot (edges are dst-sorted).
      * node_score factored: segsum(cnt) via DVE strided reduce ->
        block-diagonal fp16 redx, then ONE matmul ns^T = w2^T @ redx.
      * out window = new_embedding * 1/node_score (DVE), F-major.
  - cnt is fed pre-transposed fp16 and "half-packed" to [128, ...] so DMA
    uses all 128 partitions: partition (half*64 + c) holds cnt^T[c] for
    the window's half-th group of 2048 edges.
  - PE-row-position rule: a PSUM bank must only ever be written by matmuls
    whose stationary operand sits at one SBUF base partition. All matmuls
    here use base partition 0. (Mixing positions in one bank crashes the
    device; so does gpsimd.partition_all_reduce, and so does a dma_gather
    of more than ring-capacity indices.)
"""

import sys

if "/opt/trn_rl_repo" not in sys.path:
    sys.path.insert(0, "/opt/trn_rl_repo")

import numpy as np

# Problem sizes (fixed by the spec).
N_NODES = 20000
N_EDGES = 640000
C = 64
F = 128
N_CORES = 8
NPC = N_NODES // N_CORES       # 2500 nodes per core
EPC = N_EDGES // N_CORES       # 80000 edges per core
DEG = N_EDGES // N_NODES       # 32 edges per node
WIN_NODES = 128                # nodes per window
EPW = WIN_NODES * DEG          # 4096 edges per window
N_WIN = -(-NPC // WIN_NODES)   # 20 windows per core (last partial: 68 nodes)
PAD_EPC = N_WIN * EPW          # 81920 padded edges per core
HALF = EPW // 2                # 2048
GATHER_CHUNK = 1024            # max idxs per dma_gather (SWDGE ring capacity)
N_QUEUES = 4                   # SWDGE queues; rotate gathers across them

_CACHE = {}


def _build_nc(n_win=N_WIN, skip=()):
    import concourse.bass as bass  # noqa: F401
    import concourse.bacc as bacc
    import concourse.tile as tile
    import concourse.mybir as mybir
    from contextlib import ExitStack

    f32 = mybir.dt.float32
    f16 = mybir.dt.float16
    i16 = mybir.dt.int16
    AF = mybir.ActivationFunctionType
    AX = mybir.AxisListType

    nc = bacc.Bacc("TRN2", target_bir_lowering=False, debug=False,
                   num_swdge_queues=N_QUEUES)
    cntp = nc.declare_dram_parameter("cntp", [128, N_WIN * HALF], f16, isOutput=False)
    idx = nc.declare_dram_parameter("idx", [128, PAD_EPC // 16], i16, isOutput=False)
    emb = nc.declare_dram_parameter("emb", [N_NODES, F], f32, isOutput=False)
    imp = nc.declare_dram_parameter("imp", [C, F], f32, isOutput=False)
    pbase = nc.declare_dram_parameter("pbase", [128, 4], f16, isOutput=False)
    out = nc.declare_dram_parameter("out", [F, NPC], f32, isOutput=True)

    with ExitStack() as ctx:
        tc = ctx.enter_context(tile.TileContext(nc))
        const = ctx.enter_context(tc.tile_pool(name="const", bufs=1))

        # ---- constants ----
        pbase_sb = const.tile([128, 4], f16)
        nc.sync.dma_start(pbase_sb[:], pbase[:, :])

        # ---- softmax(importance, axis=0) on DVE (block transposes) ----
        imp_sb = const.tile([C, F], f32)
        nc.sync.dma_start(imp_sb[:], imp[:, :])
        exp_sb = const.tile([C, F], f32)
        nc.scalar.activation(exp_sb[:], imp_sb[:], AF.Exp)
        expT_sb = const.tile([128, C], f32)
        for i in range(C // 32):
            for j in range(F // 32):
                nc.vector.transpose(
                    expT_sb[32 * j:32 * j + 32, 32 * i:32 * i + 32],
                    exp_sb[32 * i:32 * i + 32, 32 * j:32 * j + 32])
        s_sb = const.tile([128, 1], f32)
        nc.vector.reduce_sum(s_sb[:], expT_sb[:], axis=AX.X)
        rec_sb = const.tile([128, 1], f32)
        nc.vector.reciprocal(rec_sb[:], s_sb[:])
        wT_sb = const.tile([128, C], f32)
        nc.vector.tensor_scalar_mul(wT_sb[:], expT_sb[:], rec_sb[:])
        w2_sb = const.tile([128, F], f32)
        for i in range(F // 32):
            for j in range(C // 32):
                nc.vector.transpose(
                    w2_sb[32 * j:32 * j + 32, 32 * i:32 * i + 32],
                    wT_sb[32 * i:32 * i + 32, 32 * j:32 * j + 32])
        nc.scalar.dma_start(w2_sb[C:128, :], w2_sb[0:C, :])

        # fp16 block-diagonal Wstack for the paired edge-score matmuls.
        wstack = const.tile([128, 2 * F], f16)
        nc.vector.memset(wstack[:], 0.0)
        nc.vector.tensor_copy(wstack[0:C, 0:F], w2_sb[0:C, :])
        nc.vector.tensor_copy(wstack[C:128, F:2 * F], w2_sb[C:128, :])

        out_sb = const.tile([128, NPC], f32)

        cnt_pool = ctx.enter_context(tc.tile_pool(name="cnt", bufs=2))
        idx_pool = ctx.enter_context(tc.tile_pool(name="idx", bufs=3))
        gath_pool = ctx.enter_context(tc.tile_pool(name="gath", bufs=4))
        es_pool = ctx.enter_context(tc.tile_pool(name="es", bufs=3, space="PSUM"))
        msgs_pool = ctx.enter_context(tc.tile_pool(name="msgs", bufs=3))
        ne_pool = ctx.enter_context(tc.tile_pool(name="ne", bufs=2, space="PSUM"))
        ns_pool = ctx.enter_context(tc.tile_pool(name="ns", bufs=1, space="PSUM"))
        red_pool = ctx.enter_context(tc.tile_pool(name="red", bufs=2))
        rtree_pool = ctx.enter_context(tc.tile_pool(name="rtree", bufs=2))
        rns_pool = ctx.enter_context(tc.tile_pool(name="rns", bufs=2))

        gq = 0  # rotating SWDGE queue index

        # ---- main loop over 128-node windows ----
        # Edges are padded to whole windows with cnt=0 / idx=0, so every
        # window runs the full 32 tiles; only the final column copies are
        # restricted to the window's real node count.
        for w in range(n_win):
            nodes_w = min(WIN_NODES, NPC - w * WIN_NODES)
            nt = (nodes_w * DEG) // 128      # real edge tiles (32; last: 17)
            n_idx = nt * 128

            cnt_sb = cnt_pool.tile([128, HALF], f16, tag="cnt")
            nc.sync.dma_start(cnt_sb[:], cntp[:, w * HALF:(w + 1) * HALF])

            idx_sb = idx_pool.tile([128, EPW // 16], i16, tag="idx")
            nc.sync.dma_start(
                idx_sb[:, :n_idx // 16],
                idx[:, w * (EPW // 16): w * (EPW // 16) + n_idx // 16])

            gath = gath_pool.tile([128, EPW], f32, tag="gath")
            if "gather" in skip:
                nc.vector.memset(gath[:], 1.0)
            else:
                gath3 = gath[:].rearrange("p (t f) -> p t f", f=F)
                for e0 in range(0, n_idx, GATHER_CHUNK):
                    ecnt = min(GATHER_CHUNK, n_idx - e0)
                    nc.gpsimd.dma_gather(
                        out_ap=gath3[:, e0 // 128:(e0 + ecnt) // 128, :],
                        in_ap=emb[:, :],
                        idxs_ap=idx_sb[:, e0 // 16:(e0 + ecnt) // 16],
                        num_idxs=ecnt,
                        num_idxs_reg=ecnt,
                        elem_size=F,
                        queue_num=gq,
                    )
                    gq = (gq + 1) % N_QUEUES

            ne_ps = ne_pool.tile([128, 128], f32, tag="ne")
            g3 = gath[:].rearrange("p (t f) -> p t f", f=F)
            if nt == 32:
                # two tile pairs (j, j+16), (j+1, j+17) per PSUM bank; one
                # [128, 512] DVE multiply for all four tiles.
                for j in range(0, 16, 2):
                    es_ps = es_pool.tile([128, 512], f32, tag="es")
                    nc.tensor.matmul(
                        es_ps[:, 0:2 * F], cnt_sb[:, 128 * j:128 * (j + 1)],
                        wstack[:], start=True, stop=True)
                    nc.tensor.matmul(
                        es_ps[:, 2 * F:4 * F],
                        cnt_sb[:, 128 * (j + 1):128 * (j + 2)],
                        wstack[:], start=True, stop=True)
                    msgs = msgs_pool.tile([128, 512], f16, tag="msgs")
                    g4 = gath[:].rearrange(
                        "p (h j f) -> p j h f", h=2, f=F)  # t = h*16 + j
                    nc.vector.tensor_mul(
                        msgs[:].rearrange("p (j h f) -> p j h f", j=2, f=F),
                        g4[:, j:j + 2, :, :],
                        es_ps[:].rearrange("p (j h f) -> p j h f", j=2, f=F),
                    )
                    for a, t in enumerate((j, j + 16, j + 1, j + 17)):
                        nc.tensor.matmul(
                            ne_ps[:, 4 * t:4 * t + 4],
                            msgs[:, a * F:(a + 1) * F], pbase_sb[:],
                            start=True, stop=True,
                        )
            else:
                for j in range(min(nt, 16)):
                    has_hi = j + 16 < nt
                    es_ps = es_pool.tile([128, 512], f32, tag="es")
                    nw = 2 * F if has_hi else F
                    nc.tensor.matmul(
                        es_ps[:, :nw], cnt_sb[:, 128 * j:128 * (j + 1)],
                        wstack[:, :nw],
                        start=True, stop=True,
                    )
                    msgs = msgs_pool.tile([128, 512], f16, tag="msgs")
                    if has_hi:
                        nc.vector.tensor_mul(
                            msgs[:, :2 * F].rearrange("p (t f) -> p t f", f=F),
                            g3[:, j:j + 17:16, :],
                            es_ps[:, :2 * F].rearrange("p (t f) -> p t f", f=F),
                        )
                    else:
                        nc.vector.tensor_mul(
                            msgs[:, 0:F], g3[:, j, :], es_ps[:, 0:F])
                    nc.tensor.matmul(
                        ne_ps[:, 4 * j:4 * j + 4],
                        msgs[:, 0:F], pbase_sb[:],
                        start=True, stop=True,
                    )
                    if has_hi:
                        nc.tensor.matmul(
                            ne_ps[:, 64 + 4 * j:64 + 4 * j + 4],
                            msgs[:, F:2 * F], pbase_sb[:],
                            start=True, stop=True,
                        )

            # node_score path: segsum(cnt) on DVE, then ns^T = W^T @ segsum.
            # Two f32 matmuls in position-dedicated PSUM banks (lo: PE rows
            # 0-63, hi: rows 64-127) to honor the PE-row-position rule.
            # (NB: running this as a GpSimd add tree measured 6.5x SLOWER
            # end-to-end - Q7 elementwise ops stall the gather pipeline.)
            red_sb = red_pool.tile([128, 64], f32, tag="red")
            cnt3 = cnt_sb[:].rearrange("p (g d) -> p g d", d=DEG)
            nc.vector.reduce_sum(red_sb[:], cnt3, axis=AX.X)
            lo_w = min(64, nodes_w)
            hi_w = nodes_w - lo_w
            ns_lo = ns_pool.tile([128, 64], f32, tag="ns_lo")
            nc.tensor.matmul(ns_lo[:], w2_sb[0:64, :], red_sb[0:64, :],
                             start=True, stop=True)
            rns_sb = rns_pool.tile([128, 128], f32, tag="rns")
            nc.vector.reciprocal_approx_fast(rns_sb[:, :lo_w], ns_lo[:, :lo_w])
            if hi_w > 0:
                ns_hi = ns_pool.tile([128, 64], f32, tag="ns_hi")
                nc.tensor.matmul(ns_hi[:], w2_sb[64:128, :], red_sb[64:128, :],
                                 start=True, stop=True)
                nc.vector.reciprocal_approx_fast(rns_sb[:, 64:64 + hi_w], ns_hi[:, :hi_w])
            nc.vector.tensor_mul(
                out_sb[:, w * WIN_NODES:w * WIN_NODES + nodes_w],
                ne_ps[:, :nodes_w],
                rns_sb[:, :nodes_w],
            )

        nc.sync.dma_start(out[:, :], out_sb[:, :NPC])

    nc.compile()
    return nc


def get_nc():
    if "nc" not in _CACHE:
        _CACHE["nc"] = _build_nc()
    return _CACHE["nc"]


def prep_in_maps(inputs):
    cnt = np.asarray(inputs["cnt"], dtype=np.float32)
    emb = np.ascontiguousarray(np.asarray(inputs["embedding"], dtype=np.float32))
    imp = np.ascontiguousarray(np.asarray(inputs["importance"], dtype=np.float32))
    src = np.asarray(inputs["src"], dtype=np.int64)
    dst = np.asarray(inputs["dst"], dtype=np.int64)

    perm = np.argsort(dst, kind="stable")
    src_s = src[perm]
    cnt_s = cnt[perm].astype(np.float16)

    pbase = np.zeros((128, 4), np.float16)
    pbase[np.arange(128), np.arange(128) // DEG] = 1.0

    in_maps = []
    for c in range(N_CORES):
        sl = slice(c * EPC, (c + 1) * EPC)
        cnt_core = np.zeros((PAD_EPC, C), np.float16)
        cnt_core[:EPC] = cnt_s[sl]
        src_core = np.zeros((PAD_EPC,), np.int64)
        src_core[:EPC] = src_s[sl]
        # half-pack: [w, half, j, c] -> [half*64+c, w*HALF+j]
        cc = cnt_core.reshape(N_WIN, 2, HALF, C)
        cntp = np.ascontiguousarray(
            cc.transpose(1, 3, 0, 2).reshape(128, N_WIN * HALF))
        # wrapped int16 index layout: idx i at [i%16, i//16], replicated x8
        idxw = np.ascontiguousarray(
            np.tile(src_core.reshape(PAD_EPC // 16, 16).T, (8, 1)).astype(np.int16))
        in_maps.append({
            "cntp": cntp,
            "idx": idxw,
            "emb": emb,
            "imp": imp,
            "pbase": pbase,
        })
    return in_maps


def unshard(core_outs):
    # each core out: [F, NPC] (F-major); concat over node axis, transpose.
    full = np.concatenate(core_outs, axis=1)          # [F, N]
    return np.ascontiguousarray(full.T.astype(np.float32))


def run(inputs, trace=False):
    from concourse.bass_utils import run_bass_kernel_spmd

    nc = get_nc()
    in_maps = prep_in_maps(inputs)
    res = run_bass_kernel_spmd(
        nc, in_maps, core_ids=list(range(N_CORES)), trace=trace)
    outs = [res.results[i]["out"] for i in range(N_CORES)]
    return unshard(outs), res


def kernel(**inputs):
    out, _ = run(inputs, trace=False)
    return out


# revision 39
# speedup vs baseline: 6.7216x; 1.0091x over previous
"""Trainium2 Bass kernel for nn_FOGCNConv (GNN message passing).

Math (reference):
    weight = softmax(importance, axis=0)            # [C, F]
    edge_score = cnt @ weight                       # [E, F]
    msgs = embedding[src] * edge_score              # [E, F]
    new_embedding = segment_sum(msgs, dst, N)       # [N, F]
    node_score = segment_sum(edge_score, dst, N)    # [N, F]
    out = new_embedding / node_score

Key structural facts (hardcoded; guaranteed by the input spec):
    N=20000 nodes, E=640000 edges, C=64, F=128, and dst is a permutation of
    arange(E) % N  =>  every node has exactly DEG=32 incoming edges.

Strategy:
  - Host: sort edges by dst. Then the two segment-sums become perfectly
    regular reductions over groups of 32 consecutive edges.
  - Shard *contiguous dst ranges* across the 8 cores (2500 nodes / 80000
    edges each) => fully independent cores, no collectives.
  - Device (per core, per 128-node window = 4096 edges = 32 edge tiles):
      * dma_gather embedding rows for the window's src indices
        (edge e -> partition e%128, free block e//128), 1024 idxs per
        gather (SWDGE ring capacity), rotated over 4 SWDGE queues.
      * edge_score for tile pair (j, j+16) in ONE K=128 fp16 matmul:
        lhsT = packed cnt^T [128, 128] (c-halves stacked), rhs = Wstack
        [128, 256] block-diagonal softmax weights -> es [128e, 256] f32.
      * msgs = gathered * edge_score on DVE (one 3D-strided op per pair),
        cast to fp16.
      * new_embedding window [128f, 128n] via PE "segment matmuls": per
        edge tile t, out[:, 4t:4t+4] = msgs_t^T @ P where P[e, j] =
        (e//32 == j) is a constant one-hot (edges are dst-sorted).
      * node_score factored: segsum(cnt) via DVE strided reduce ->
        block-diagonal fp16 redx, then ONE matmul ns^T = w2^T @ redx.
      * out window = new_embedding * 1/node_score (DVE), F-major.
  - cnt is fed pre-transposed fp16 and "half-packed" to [128, ...] so DMA
    uses all 128 partitions: partition (half*64 + c) holds cnt^T[c] for
    the window's half-th group of 2048 edges.
  - PE-row-position rule: a PSUM bank must only ever be written by matmuls
    whose stationary operand sits at one SBUF base partition. All matmuls
    here use base partition 0. (Mixing positions in one bank crashes the
    device; so does gpsimd.partition_all_reduce, and so does a dma_gather
    of more than ring-capacity indices.)
"""

import sys

if "/opt/trn_rl_repo" not in sys.path:
    sys.path.insert(0, "/opt/trn_rl_repo")

import numpy as np

# Problem sizes (fixed by the spec).
N_NODES = 20000
N_EDGES = 640000
C = 64
F = 128
N_CORES = 8
NPC = N_NODES // N_CORES       # 2500 nodes per core
EPC = N_EDGES // N_CORES       # 80000 edges per core
DEG = N_EDGES // N_NODES       # 32 edges per node
WIN_NODES = 128                # nodes per window
EPW = WIN_NODES * DEG          # 4096 edges per window
N_WIN = -(-NPC // WIN_NODES)   # 20 windows per core (last partial: 68 nodes)
PAD_EPC = N_WIN * EPW          # 81920 padded edges per core
HALF = EPW // 2                # 2048
GATHER_CHUNK = 1024            # max idxs per dma_gather (SWDGE ring capacity)
N_QUEUES = 4                   # SWDGE queues; rotate gathers across them

_CACHE = {}


def _build_nc(n_win=N_WIN, skip=()):
    import concourse.bass as bass  # noqa: F401
    import concourse.bacc as bacc
    import concourse.tile as tile
    import concourse.mybir as mybir
    from contextlib import ExitStack

    f32 = mybir.dt.float32
    f16 = mybir.dt.float16
    i16 = mybir.dt.int16
    AF = mybir.ActivationFunctionType
    AX = mybir.AxisListType

    nc = bacc.Bacc("TRN2", target_bir_lowering=False, debug=False,
                   num_swdge_queues=N_QUEUES)
    cntp = nc.declare_dram_parameter("cntp", [128, N_WIN * HALF], f16, isOutput=False)
    idx = nc.declare_dram_parameter("idx", [128, PAD_EPC // 16], i16, isOutput=False)
    emb = nc.declare_dram_parameter("emb", [N_NODES, F], f32, isOutput=False)
    imp = nc.declare_dram_parameter("imp", [C, F], f32, isOutput=False)
    pbase = nc.declare_dram_parameter("pbase", [128, 4], f16, isOutput=False)
    out = nc.declare_dram_parameter("out", [F, NPC], f32, isOutput=True)

    with ExitStack() as ctx:
        tc = ctx.enter_context(tile.TileContext(nc))
        const = ctx.enter_context(tc.tile_pool(name="const", bufs=1))

        # ---- constants ----
        pbase_sb = const.tile([128, 4], f16)
        nc.sync.dma_start(pbase_sb[:], pbase[:, :])

        # ---- softmax(importance, axis=0) on DVE (block transposes) ----
        imp_sb = const.tile([C, F], f32)
        nc.sync.dma_start(imp_sb[:], imp[:, :])
        exp_sb = const.tile([C, F], f32)
        nc.scalar.activation(exp_sb[:], imp_sb[:], AF.Exp)
        expT_sb = const.tile([128, C], f32)
        for i in range(C // 32):
            for j in range(F // 32):
                nc.vector.transpose(
                    expT_sb[32 * j:32 * j + 32, 32 * i:32 * i + 32],
                    exp_sb[32 * i:32 * i + 32, 32 * j:32 * j + 32])
        s_sb = const.tile([128, 1], f32)
        nc.vector.reduce_sum(s_sb[:], expT_sb[:], axis=AX.X)
        rec_sb = const.tile([128, 1], f32)
        nc.vector.reciprocal(rec_sb[:], s_sb[:])
        wT_sb = const.tile([128, C], f32)
        nc.vector.tensor_scalar_mul(wT_sb[:], expT_sb[:], rec_sb[:])
        w2_sb = const.tile([128, F], f32)
        for i in range(F // 32):
            for j in range(C // 32):
                nc.vector.transpose(
                    w2_sb[32 * j:32 * j + 32, 32 * i:32 * i + 32],
                    wT_sb[32 * i:32 * i + 32, 32 * j:32 * j + 32])
        nc.scalar.dma_start(w2_sb[C:128, :], w2_sb[0:C, :])

        # fp16 block-diagonal Wstack for the paired edge-score matmuls.
        wstack = const.tile([128, 2 * F], f16)
        nc.vector.memset(wstack[:], 0.0)
        nc.vector.tensor_copy(wstack[0:C, 0:F], w2_sb[0:C, :])
        nc.vector.tensor_copy(wstack[C:128, F:2 * F], w2_sb[C:128, :])

        out_sb = const.tile([128, NPC], f32)

        cnt_pool = ctx.enter_context(tc.tile_pool(name="cnt", bufs=2))
        idx_pool = ctx.enter_context(tc.tile_pool(name="idx", bufs=3))
        gath_pool = ctx.enter_context(tc.tile_pool(name="gath", bufs=5))
        es_pool = ctx.enter_context(tc.tile_pool(name="es", bufs=4, space="PSUM"))
        msgs_pool = ctx.enter_context(tc.tile_pool(name="msgs", bufs=4))
        ne_pool = ctx.enter_context(tc.tile_pool(name="ne", bufs=2, space="PSUM"))
        ns_pool = ctx.enter_context(tc.tile_pool(name="ns", bufs=1, space="PSUM"))
        red_pool = ctx.enter_context(tc.tile_pool(name="red", bufs=2))
        rns_pool = ctx.enter_context(tc.tile_pool(name="rns", bufs=2))

        gq = 0  # rotating SWDGE queue index

        # ---- main loop over 128-node windows ----
        # Edges are padded to whole windows with cnt=0 / idx=0, so every
        # window runs the full 32 tiles; only the final column copies are
        # restricted to the window's real node count.
        for w in range(n_win):
            nodes_w = min(WIN_NODES, NPC - w * WIN_NODES)
            nt = (nodes_w * DEG) // 128      # real edge tiles (32; last: 17)
            n_idx = nt * 128

            cnt_sb = cnt_pool.tile([128, HALF], f16, tag="cnt")
            nc.sync.dma_start(cnt_sb[:], cntp[:, w * HALF:(w + 1) * HALF])

            idx_sb = idx_pool.tile([128, EPW // 16], i16, tag="idx")
            nc.sync.dma_start(
                idx_sb[:, :n_idx // 16],
                idx[:, w * (EPW // 16): w * (EPW // 16) + n_idx // 16])

            gath = gath_pool.tile([128, EPW], f32, tag="gath")
            if "gather" in skip:
                nc.vector.memset(gath[:], 1.0)
            else:
                gath3 = gath[:].rearrange("p (t f) -> p t f", f=F)
                for e0 in range(0, n_idx, GATHER_CHUNK):
                    ecnt = min(GATHER_CHUNK, n_idx - e0)
                    nc.gpsimd.dma_gather(
                        out_ap=gath3[:, e0 // 128:(e0 + ecnt) // 128, :],
                        in_ap=emb[:, :],
                        idxs_ap=idx_sb[:, e0 // 16:(e0 + ecnt) // 16],
                        num_idxs=ecnt,
                        num_idxs_reg=ecnt,
                        elem_size=F,
                        queue_num=gq,
                    )
                    gq = (gq + 1) % N_QUEUES

            ne_ps = ne_pool.tile([128, 128], f32, tag="ne")
            g3 = gath[:].rearrange("p (t f) -> p t f", f=F)
            if nt == 32:
                # two tile pairs (j, j+16), (j+1, j+17) per PSUM bank; one
                # [128, 512] DVE multiply for all four tiles.
                for j in range(0, 16, 2):
                    es_ps = es_pool.tile([128, 512], f32, tag="es")
                    nc.tensor.matmul(
                        es_ps[:, 0:2 * F], cnt_sb[:, 128 * j:128 * (j + 1)],
                        wstack[:], start=True, stop=True)
                    nc.tensor.matmul(
                        es_ps[:, 2 * F:4 * F],
                        cnt_sb[:, 128 * (j + 1):128 * (j + 2)],
                        wstack[:], start=True, stop=True)
                    msgs = msgs_pool.tile([128, 512], f16, tag="msgs")
                    g4 = gath[:].rearrange(
                        "p (h j f) -> p j h f", h=2, f=F)  # t = h*16 + j
                    nc.vector.tensor_mul(
                        msgs[:].rearrange("p (j h f) -> p j h f", j=2, f=F),
                        g4[:, j:j + 2, :, :],
                        es_ps[:].rearrange("p (j h f) -> p j h f", j=2, f=F),
                    )
                    for a, t in enumerate((j, j + 16, j + 1, j + 17)):
                        nc.tensor.matmul(
                            ne_ps[:, 4 * t:4 * t + 4],
                            msgs[:, a * F:(a + 1) * F], pbase_sb[:],
                            start=True, stop=True,
                        )
            else:
                for j in range(min(nt, 16)):
                    has_hi = j + 16 < nt
                    es_ps = es_pool.tile([128, 512], f32, tag="es")
                    nw = 2 * F if has_hi else F
                    nc.tensor.matmul(
                        es_ps[:, :nw], cnt_sb[:, 128 * j:128 * (j + 1)],
                        wstack[:, :nw],
                        start=True, stop=True,
                    )
                    msgs = msgs_pool.tile([128, 512], f16, tag="msgs")
                    if has_hi:
                        nc.vector.tensor_mul(
                            msgs[:, :2 * F].rearrange("p (t f) -> p t f", f=F),
                            g3[:, j:j + 17:16, :],
                            es_ps[:, :2 * F].rearrange("p (t f) -> p t f", f=F),
                        )
                    else:
                        nc.vector.tensor_mul(
                            msgs[:, 0:F], g3[:, j, :], es_ps[:, 0:F])
                    nc.tensor.matmul(
                        ne_ps[:, 4 * j:4 * j + 4],
                        msgs[:, 0:F], pbase_sb[:],
                        start=True, stop=True,
                    )
                    if has_hi:
                        nc.tensor.matmul(
                            ne_ps[:, 64 + 4 * j:64 + 4 * j + 4],
                            msgs[:, F:2 * F], pbase_sb[:],
                            start=True, stop=True,
                        )

            # node_score path: segsum(cnt) on DVE, then ns^T = W^T @ segsum.
            # Two f32 matmuls in position-dedicated PSUM banks (lo: PE rows
            # 0-63, hi: rows 64-127) to honor the PE-row-position rule.
            # (NB: running this as a GpSimd add tree measured 6.5x SLOWER
            # end-to-end - Q7 elementwise ops stall the gather pipeline.)
            red_sb = red_pool.tile([128, 64], f32, tag="red")
            cnt3 = cnt_sb[:].rearrange("p (g d) -> p g d", d=DEG)
            nc.vector.reduce_sum(red_sb[:], cnt3, axis=AX.X)
            lo_w = min(64, nodes_w)
            hi_w = nodes_w - lo_w
            ns_lo = ns_pool.tile([128, 64], f32, tag="ns_lo")
            nc.tensor.matmul(ns_lo[:], w2_sb[0:64, :], red_sb[0:64, :],
                             start=True, stop=True)
            rns_sb = rns_pool.tile([128, 128], f32, tag="rns")
            nc.vector.reciprocal_approx_fast(rns_sb[:, :lo_w], ns_lo[:, :lo_w])
            if hi_w > 0:
                ns_hi = ns_pool.tile([128, 64], f32, tag="ns_hi")
                nc.tensor.matmul(ns_hi[:], w2_sb[64:128, :], red_sb[64:128, :],
                                 start=True, stop=True)
                nc.vector.reciprocal_approx_fast(rns_sb[:, 64:64 + hi_w], ns_hi[:, :hi_w])
            nc.vector.tensor_mul(
                out_sb[:, w * WIN_NODES:w * WIN_NODES + nodes_w],
                ne_ps[:, :nodes_w],
                rns_sb[:, :nodes_w],
            )

        nc.sync.dma_start(out[:, :], out_sb[:, :NPC])

    nc.compile()
    return nc


def get_nc():
    if "nc" not in _CACHE:
        _CACHE["nc"] = _build_nc()
    return _CACHE["nc"]


def prep_in_maps(inputs):
    cnt = np.asarray(inputs["cnt"], dtype=np.float32)
    emb = np.ascontiguousarray(np.asarray(inputs["embedding"], dtype=np.float32))
    imp = np.ascontiguousarray(np.asarray(inputs["importance"], dtype=np.float32))
    src = np.asarray(inputs["src"], dtype=np.int64)
    dst = np.asarray(inputs["dst"], dtype=np.int64)

    perm = np.argsort(dst, kind="stable")
    src_s = src[perm]
    cnt_s = cnt[perm].astype(np.float16)

    pbase = np.zeros((128, 4), np.float16)
    pbase[np.arange(128), np.arange(128) // DEG] = 1.0

    in_maps = []
    for c in range(N_CORES):
        sl = slice(c * EPC, (c + 1) * EPC)
        cnt_core = np.zeros((PAD_EPC, C), np.float16)
        cnt_core[:EPC] = cnt_s[sl]
        src_core = np.zeros((PAD_EPC,), np.int64)
        src_core[:EPC] = src_s[sl]
        # half-pack: [w, half, j, c] -> [half*64+c, w*HALF+j]
        cc = cnt_core.reshape(N_WIN, 2, HALF, C)
        cntp = np.ascontiguousarray(
            cc.transpose(1, 3, 0, 2).reshape(128, N_WIN * HALF))
        # wrapped int16 index layout: idx i at [i%16, i//16], replicated x8
        idxw = np.ascontiguousarray(
            np.tile(src_core.reshape(PAD_EPC // 16, 16).T, (8, 1)).astype(np.int16))
        in_maps.append({
            "cntp": cntp,
            "idx": idxw,
            "emb": emb,
            "imp": imp,
            "pbase": pbase,
        })
    return in_maps


def unshard(core_outs):
    # each core out: [F, NPC] (F-major); concat over node axis, transpose.
    full = np.concatenate(core_outs, axis=1)          # [F, N]
    return np.ascontiguousarray(full.T.astype(np.float32))


def run(inputs, trace=False):
    from concourse.bass_utils import run_bass_kernel_spmd

    nc = get_nc()
    in_maps = prep_in_maps(inputs)
    res = run_bass_kernel_spmd(
        nc, in_maps, core_ids=list(range(N_CORES)), trace=trace)
    outs = [res.results[i]["out"] for i in range(N_CORES)]
    return unshard(outs), res


def kernel(**inputs):
    out, _ = run(inputs, trace=False)
    return out


# revision 40
# speedup vs baseline: 7.4041x; 1.1015x over previous
"""Trainium2 Bass kernel for nn_FOGCNConv (GNN message passing).

Math (reference):
    weight = softmax(importance, axis=0)            # [C, F]
    edge_score = cnt @ weight                       # [E, F]
    msgs = embedding[src] * edge_score              # [E, F]
    new_embedding = segment_sum(msgs, dst, N)       # [N, F]
    node_score = segment_sum(edge_score, dst, N)    # [N, F]
    out = new_embedding / node_score

Key structural facts (hardcoded; guaranteed by the input spec):
    N=20000 nodes, E=640000 edges, C=64, F=128, and dst is a permutation of
    arange(E) % N  =>  every node has exactly DEG=32 incoming edges.

Strategy:
  - Host: sort edges by dst. Then the two segment-sums become perfectly
    regular reductions over groups of 32 consecutive edges.
  - Shard *contiguous dst ranges* across the 8 cores (2500 nodes / 80000
    edges each) => fully independent cores, no collectives.
  - Device (per core, per 128-node window = 4096 edges = 32 edge tiles):
      * dma_gather embedding rows for the window's src indices
        (edge e -> partition e%128, free block e//128), 1024 idxs per
        gather (SWDGE ring capacity), rotated over 4 SWDGE queues.
      * edge_score for tile pair (j, j+16) in ONE K=128 fp16 matmul:
        lhsT = packed cnt^T [128, 128] (c-halves stacked), rhs = Wstack
        [128, 256] block-diagonal softmax weights -> es [128e, 256] f32.
      * msgs = gathered * edge_score on DVE (one 3D-strided op per pair),
        cast to fp16.
      * new_embedding window [128f, 128n] via PE "segment matmuls": per
        edge tile t, out[:, 4t:4t+4] = msgs_t^T @ P where P[e, j] =
        (e//32 == j) is a constant one-hot (edges are dst-sorted).
      * node_score factored: segsum(cnt) via DVE strided reduce ->
        block-diagonal fp16 redx, then ONE matmul ns^T = w2^T @ redx.
      * out window = new_embedding * 1/node_score (DVE), F-major.
  - cnt is fed pre-transposed fp16 and "half-packed" to [128, ...] so DMA
    uses all 128 partitions: partition (half*64 + c) holds cnt^T[c] for
    the window's half-th group of 2048 edges.
  - PE-row-position rule: a PSUM bank must only ever be written by matmuls
    whose stationary operand sits at one SBUF base partition. All matmuls
    here use base partition 0. (Mixing positions in one bank crashes the
    device; so does gpsimd.partition_all_reduce, and so does a dma_gather
    of more than ring-capacity indices.)
"""

import sys

if "/opt/trn_rl_repo" not in sys.path:
    sys.path.insert(0, "/opt/trn_rl_repo")

import numpy as np

# Problem sizes (fixed by the spec).
N_NODES = 20000
N_EDGES = 640000
C = 64
F = 128
N_CORES = 8
NPC = N_NODES // N_CORES       # 2500 nodes per core
EPC = N_EDGES // N_CORES       # 80000 edges per core
DEG = N_EDGES // N_NODES       # 32 edges per node
WIN_NODES = 128                # nodes per window
EPW = WIN_NODES * DEG          # 4096 edges per window
N_WIN = -(-NPC // WIN_NODES)   # 20 windows per core (last partial: 68 nodes)
PAD_EPC = N_WIN * EPW          # 81920 padded edges per core
HALF = EPW // 2                # 2048
GATHER_CHUNK = 1024            # max idxs per dma_gather (SWDGE ring capacity)
N_QUEUES = 4                   # SWDGE queues; rotate gathers across them

_CACHE = {}


def _build_nc(n_win=N_WIN, skip=()):
    import concourse.bass as bass  # noqa: F401
    import concourse.bacc as bacc
    import concourse.tile as tile
    import concourse.mybir as mybir
    from contextlib import ExitStack

    f32 = mybir.dt.float32
    f16 = mybir.dt.float16
    i16 = mybir.dt.int16
    AF = mybir.ActivationFunctionType
    AX = mybir.AxisListType

    nc = bacc.Bacc("TRN2", target_bir_lowering=False, debug=False,
                   num_swdge_queues=N_QUEUES)
    cntp = nc.declare_dram_parameter("cntp", [128, N_WIN * HALF], f16, isOutput=False)
    idx = nc.declare_dram_parameter("idx", [128, PAD_EPC // 16], i16, isOutput=False)
    emb = nc.declare_dram_parameter("emb", [N_NODES, F], f32, isOutput=False)
    imp = nc.declare_dram_parameter("imp", [C, F], f32, isOutput=False)
    pbase = nc.declare_dram_parameter("pbase", [128, 4], f16, isOutput=False)
    out = nc.declare_dram_parameter("out", [F, NPC], f32, isOutput=True)

    with ExitStack() as ctx:
        tc = ctx.enter_context(tile.TileContext(nc))
        const = ctx.enter_context(tc.tile_pool(name="const", bufs=1))

        # ---- constants ----
        pbase_sb = const.tile([128, 4], f16)
        nc.sync.dma_start(pbase_sb[:], pbase[:, :])

        # ---- softmax(importance, axis=0) on DVE (block transposes) ----
        imp_sb = const.tile([C, F], f32)
        nc.sync.dma_start(imp_sb[:], imp[:, :])
        exp_sb = const.tile([C, F], f32)
        nc.scalar.activation(exp_sb[:], imp_sb[:], AF.Exp)
        expT_sb = const.tile([128, C], f32)
        for i in range(C // 32):
            for j in range(F // 32):
                nc.vector.transpose(
                    expT_sb[32 * j:32 * j + 32, 32 * i:32 * i + 32],
                    exp_sb[32 * i:32 * i + 32, 32 * j:32 * j + 32])
        s_sb = const.tile([128, 1], f32)
        nc.vector.reduce_sum(s_sb[:], expT_sb[:], axis=AX.X)
        rec_sb = const.tile([128, 1], f32)
        nc.vector.reciprocal(rec_sb[:], s_sb[:])
        wT_sb = const.tile([128, C], f32)
        nc.vector.tensor_scalar_mul(wT_sb[:], expT_sb[:], rec_sb[:])
        w2_sb = const.tile([128, F], f32)
        for i in range(F // 32):
            for j in range(C // 32):
                nc.vector.transpose(
                    w2_sb[32 * j:32 * j + 32, 32 * i:32 * i + 32],
                    wT_sb[32 * i:32 * i + 32, 32 * j:32 * j + 32])
        nc.scalar.dma_start(w2_sb[C:128, :], w2_sb[0:C, :])

        # fp16 block-diagonal Wstack for the paired edge-score matmuls.
        wstack = const.tile([128, 2 * F], f16)
        nc.vector.memset(wstack[:], 0.0)
        nc.vector.tensor_copy(wstack[0:C, 0:F], w2_sb[0:C, :])
        nc.vector.tensor_copy(wstack[C:128, F:2 * F], w2_sb[C:128, :])

        out_sb = const.tile([128, NPC], f32)

        cnt_pool = ctx.enter_context(tc.tile_pool(name="cnt", bufs=3))
        idx_pool = ctx.enter_context(tc.tile_pool(name="idx", bufs=4))
        gath_pool = ctx.enter_context(tc.tile_pool(name="gath", bufs=5))
        es_pool = ctx.enter_context(tc.tile_pool(name="es", bufs=4, space="PSUM"))
        msgs_pool = ctx.enter_context(tc.tile_pool(name="msgs", bufs=6))
        ne_pool = ctx.enter_context(tc.tile_pool(name="ne", bufs=2, space="PSUM"))
        ns_pool = ctx.enter_context(tc.tile_pool(name="ns", bufs=1, space="PSUM"))
        red_pool = ctx.enter_context(tc.tile_pool(name="red", bufs=3))
        rns_pool = ctx.enter_context(tc.tile_pool(name="rns", bufs=3))

        gq = 0  # rotating SWDGE queue index

        # ---- main loop over 128-node windows ----
        # Edges are padded to whole windows with cnt=0 / idx=0, so every
        # window runs the full 32 tiles; only the final column copies are
        # restricted to the window's real node count.
        for w in range(n_win):
            nodes_w = min(WIN_NODES, NPC - w * WIN_NODES)
            nt = (nodes_w * DEG) // 128      # real edge tiles (32; last: 17)
            n_idx = nt * 128

            cnt_sb = cnt_pool.tile([128, HALF], f16, tag="cnt")
            nc.sync.dma_start(cnt_sb[:], cntp[:, w * HALF:(w + 1) * HALF])

            idx_sb = idx_pool.tile([128, EPW // 16], i16, tag="idx")
            nc.sync.dma_start(
                idx_sb[:, :n_idx // 16],
                idx[:, w * (EPW // 16): w * (EPW // 16) + n_idx // 16])

            gath = gath_pool.tile([128, EPW], f32, tag="gath")
            if "gather" in skip:
                nc.vector.memset(gath[:], 1.0)
            else:
                gath3 = gath[:].rearrange("p (t f) -> p t f", f=F)
                for e0 in range(0, n_idx, GATHER_CHUNK):
                    ecnt = min(GATHER_CHUNK, n_idx - e0)
                    nc.gpsimd.dma_gather(
                        out_ap=gath3[:, e0 // 128:(e0 + ecnt) // 128, :],
                        in_ap=emb[:, :],
                        idxs_ap=idx_sb[:, e0 // 16:(e0 + ecnt) // 16],
                        num_idxs=ecnt,
                        num_idxs_reg=ecnt,
                        elem_size=F,
                        queue_num=gq,
                    )
                    gq = (gq + 1) % N_QUEUES

            ne_ps = ne_pool.tile([128, 128], f32, tag="ne")
            g3 = gath[:].rearrange("p (t f) -> p t f", f=F)
            if nt == 32:
                # two tile pairs (j, j+16), (j+1, j+17) per PSUM bank; one
                # [128, 512] DVE multiply for all four tiles.
                for j in range(0, 16, 2):
                    es_ps = es_pool.tile([128, 512], f32, tag="es")
                    nc.tensor.matmul(
                        es_ps[:, 0:2 * F], cnt_sb[:, 128 * j:128 * (j + 1)],
                        wstack[:], start=True, stop=True)
                    nc.tensor.matmul(
                        es_ps[:, 2 * F:4 * F],
                        cnt_sb[:, 128 * (j + 1):128 * (j + 2)],
                        wstack[:], start=True, stop=True)
                    msgs = msgs_pool.tile([128, 512], f16, tag="msgs")
                    g4 = gath[:].rearrange(
                        "p (h j f) -> p j h f", h=2, f=F)  # t = h*16 + j
                    nc.vector.tensor_mul(
                        msgs[:].rearrange("p (j h f) -> p j h f", j=2, f=F),
                        g4[:, j:j + 2, :, :],
                        es_ps[:].rearrange("p (j h f) -> p j h f", j=2, f=F),
                    )
                    for a, t in enumerate((j, j + 16, j + 1, j + 17)):
                        nc.tensor.matmul(
                            ne_ps[:, 4 * t:4 * t + 4],
                            msgs[:, a * F:(a + 1) * F], pbase_sb[:],
                            start=True, stop=True,
                        )
            else:
                for j in range(min(nt, 16)):
                    has_hi = j + 16 < nt
                    es_ps = es_pool.tile([128, 512], f32, tag="es")
                    nw = 2 * F if has_hi else F
                    nc.tensor.matmul(
                        es_ps[:, :nw], cnt_sb[:, 128 * j:128 * (j + 1)],
                        wstack[:, :nw],
                        start=True, stop=True,
                    )
                    msgs = msgs_pool.tile([128, 512], f16, tag="msgs")
                    if has_hi:
                        nc.vector.tensor_mul(
                            msgs[:, :2 * F].rearrange("p (t f) -> p t f", f=F),
                            g3[:, j:j + 17:16, :],
                            es_ps[:, :2 * F].rearrange("p (t f) -> p t f", f=F),
                        )
                    else:
                        nc.vector.tensor_mul(
                            msgs[:, 0:F], g3[:, j, :], es_ps[:, 0:F])
                    nc.tensor.matmul(
                        ne_ps[:, 4 * j:4 * j + 4],
                        msgs[:, 0:F], pbase_sb[:],
                        start=True, stop=True,
                    )
                    if has_hi:
                        nc.tensor.matmul(
                            ne_ps[:, 64 + 4 * j:64 + 4 * j + 4],
                            msgs[:, F:2 * F], pbase_sb[:],
                            start=True, stop=True,
                        )

            # node_score path: segsum(cnt) on DVE, then ns^T = W^T @ segsum.
            # Two f32 matmuls in position-dedicated PSUM banks (lo: PE rows
            # 0-63, hi: rows 64-127) to honor the PE-row-position rule.
            # (NB: running this as a GpSimd add tree measured 6.5x SLOWER
            # end-to-end - Q7 elementwise ops stall the gather pipeline.)
            red_sb = red_pool.tile([128, 64], f32, tag="red")
            cnt3 = cnt_sb[:].rearrange("p (g d) -> p g d", d=DEG)
            nc.vector.reduce_sum(red_sb[:], cnt3, axis=AX.X)
            lo_w = min(64, nodes_w)
            hi_w = nodes_w - lo_w
            ns_lo = ns_pool.tile([128, 64], f32, tag="ns_lo")
            nc.tensor.matmul(ns_lo[:], w2_sb[0:64, :], red_sb[0:64, :],
                             start=True, stop=True)
            rns_sb = rns_pool.tile([128, 128], f32, tag="rns")
            nc.vector.reciprocal_approx_fast(rns_sb[:, :lo_w], ns_lo[:, :lo_w])
            if hi_w > 0:
                ns_hi = ns_pool.tile([128, 64], f32, tag="ns_hi")
                nc.tensor.matmul(ns_hi[:], w2_sb[64:128, :], red_sb[64:128, :],
                                 start=True, stop=True)
                nc.vector.reciprocal_approx_fast(rns_sb[:, 64:64 + hi_w], ns_hi[:, :hi_w])
            nc.vector.tensor_mul(
                out_sb[:, w * WIN_NODES:w * WIN_NODES + nodes_w],
                ne_ps[:, :nodes_w],
                rns_sb[:, :nodes_w],
            )

        nc.sync.dma_start(out[:, :], out_sb[:, :NPC])

    nc.compile()
    return nc


def get_nc():
    if "nc" not in _CACHE:
        _CACHE["nc"] = _build_nc()
    return _CACHE["nc"]


def prep_in_maps(inputs):
    cnt = np.asarray(inputs["cnt"], dtype=np.float32)
    emb = np.ascontiguousarray(np.asarray(inputs["embedding"], dtype=np.float32))
    imp = np.ascontiguousarray(np.asarray(inputs["importance"], dtype=np.float32))
    src = np.asarray(inputs["src"], dtype=np.int64)
    dst = np.asarray(inputs["dst"], dtype=np.int64)

    perm = np.argsort(dst, kind="stable")
    src_s = src[perm]
    cnt_s = cnt[perm].astype(np.float16)

    pbase = np.zeros((128, 4), np.float16)
    pbase[np.arange(128), np.arange(128) // DEG] = 1.0

    in_maps = []
    for c in range(N_CORES):
        sl = slice(c * EPC, (c + 1) * EPC)
        cnt_core = np.zeros((PAD_EPC, C), np.float16)
        cnt_core[:EPC] = cnt_s[sl]
        src_core = np.zeros((PAD_EPC,), np.int64)
        src_core[:EPC] = src_s[sl]
        # half-pack: [w, half, j, c] -> [half*64+c, w*HALF+j]
        cc = cnt_core.reshape(N_WIN, 2, HALF, C)
        cntp = np.ascontiguousarray(
            cc.transpose(1, 3, 0, 2).reshape(128, N_WIN * HALF))
        # wrapped int16 index layout: idx i at [i%16, i//16], replicated x8
        idxw = np.ascontiguousarray(
            np.tile(src_core.reshape(PAD_EPC // 16, 16).T, (8, 1)).astype(np.int16))
        in_maps.append({
            "cntp": cntp,
            "idx": idxw,
            "emb": emb,
            "imp": imp,
            "pbase": pbase,
        })
    return in_maps


def unshard(core_outs):
    # each core out: [F, NPC] (F-major); concat over node axis, transpose.
    full = np.concatenate(core_outs, axis=1)          # [F, N]
    return np.ascontiguousarray(full.T.astype(np.float32))


def run(inputs, trace=False):
    from concourse.bass_utils import run_bass_kernel_spmd

    nc = get_nc()
    in_maps = prep_in_maps(inputs)
    res = run_bass_kernel_spmd(
        nc, in_maps, core_ids=list(range(N_CORES)), trace=trace)
    outs = [res.results[i]["out"] for i in range(N_CORES)]
    return unshard(outs), res


def kernel(**inputs):
    out, _ = run(inputs, trace=False)
    return out


# revision 41
# speedup vs baseline: 7.5691x; 1.0223x over previous
"""Trainium2 Bass kernel for nn_FOGCNConv (GNN message passing).

Math (reference):
    weight = softmax(importance, axis=0)            # [C, F]
    edge_score = cnt @ weight                       # [E, F]
    msgs = embedding[src] * edge_score              # [E, F]
    new_embedding = segment_sum(msgs, dst, N)       # [N, F]
    node_score = segment_sum(edge_score, dst, N)    # [N, F]
    out = new_embedding / node_score

Key structural facts (hardcoded; guaranteed by the input spec):
    N=20000 nodes, E=640000 edges, C=64, F=128, and dst is a permutation of
    arange(E) % N  =>  every node has exactly DEG=32 incoming edges.

Strategy:
  - Host: sort edges by dst. Then the two segment-sums become perfectly
    regular reductions over groups of 32 consecutive edges.
  - Shard *contiguous dst ranges* across the 8 cores (2500 nodes / 80000
    edges each) => fully independent cores, no collectives.
  - Device (per core, per 128-node window = 4096 edges = 32 edge tiles):
      * dma_gather embedding rows for the window's src indices
        (edge e -> partition e%128, free block e//128), 1024 idxs per
        gather (SWDGE ring capacity), rotated over 4 SWDGE queues.
      * edge_score for tile pair (j, j+16) in ONE K=128 fp16 matmul:
        lhsT = packed cnt^T [128, 128] (c-halves stacked), rhs = Wstack
        [128, 256] block-diagonal softmax weights -> es [128e, 256] f32.
      * msgs = gathered * edge_score on DVE (one 3D-strided op per pair),
        cast to fp16.
      * new_embedding window [128f, 128n] via PE "segment matmuls": per
        edge tile t, out[:, 4t:4t+4] = msgs_t^T @ P where P[e, j] =
        (e//32 == j) is a constant one-hot (edges are dst-sorted).
      * node_score factored: segsum(cnt) via DVE strided reduce ->
        block-diagonal fp16 redx, then ONE matmul ns^T = w2^T @ redx.
      * out window = new_embedding * 1/node_score (DVE), F-major.
  - cnt is fed pre-transposed fp16 and "half-packed" to [128, ...] so DMA
    uses all 128 partitions: partition (half*64 + c) holds cnt^T[c] for
    the window's half-th group of 2048 edges.
  - PE-row-position rule: a PSUM bank must only ever be written by matmuls
    whose stationary operand sits at one SBUF base partition. All matmuls
    here use base partition 0. (Mixing positions in one bank crashes the
    device; so does gpsimd.partition_all_reduce, and so does a dma_gather
    of more than ring-capacity indices.)
"""

import sys

if "/opt/trn_rl_repo" not in sys.path:
    sys.path.insert(0, "/opt/trn_rl_repo")

import numpy as np

# Problem sizes (fixed by the spec).
N_NODES = 20000
N_EDGES = 640000
C = 64
F = 128
N_CORES = 8
NPC = N_NODES // N_CORES       # 2500 nodes per core
EPC = N_EDGES // N_CORES       # 80000 edges per core
DEG = N_EDGES // N_NODES       # 32 edges per node
WIN_NODES = 128                # nodes per window
EPW = WIN_NODES * DEG          # 4096 edges per window
N_WIN = -(-NPC // WIN_NODES)   # 20 windows per core (last partial: 68 nodes)
PAD_EPC = N_WIN * EPW          # 81920 padded edges per core
HALF = EPW // 2                # 2048
GATHER_CHUNK = 1024            # max idxs per dma_gather (SWDGE ring capacity)
N_QUEUES = 4                   # SWDGE queues; rotate gathers across them

_CACHE = {}


def _build_nc(n_win=N_WIN, skip=()):
    import concourse.bass as bass  # noqa: F401
    import concourse.bacc as bacc
    import concourse.tile as tile
    import concourse.mybir as mybir
    from contextlib import ExitStack

    f32 = mybir.dt.float32
    f16 = mybir.dt.float16
    i16 = mybir.dt.int16
    AF = mybir.ActivationFunctionType
    AX = mybir.AxisListType

    nc = bacc.Bacc("TRN2", target_bir_lowering=False, debug=False,
                   num_swdge_queues=N_QUEUES)
    cntp = nc.declare_dram_parameter("cntp", [128, N_WIN * HALF], f16, isOutput=False)
    idx = nc.declare_dram_parameter("idx", [128, PAD_EPC // 16], i16, isOutput=False)
    emb = nc.declare_dram_parameter("emb", [N_NODES, F], f32, isOutput=False)
    imp = nc.declare_dram_parameter("imp", [C, F], f32, isOutput=False)
    pbase = nc.declare_dram_parameter("pbase", [128, 4], f16, isOutput=False)
    out = nc.declare_dram_parameter("out", [F, NPC], f32, isOutput=True)

    with ExitStack() as ctx:
        tc = ctx.enter_context(tile.TileContext(nc))
        const = ctx.enter_context(tc.tile_pool(name="const", bufs=1))

        # ---- constants ----
        pbase_sb = const.tile([128, 4], f16)
        nc.sync.dma_start(pbase_sb[:], pbase[:, :])

        # ---- softmax(importance, axis=0) on DVE (block transposes) ----
        imp_sb = const.tile([C, F], f32)
        nc.sync.dma_start(imp_sb[:], imp[:, :])
        exp_sb = const.tile([C, F], f32)
        nc.scalar.activation(exp_sb[:], imp_sb[:], AF.Exp)
        expT_sb = const.tile([128, C], f32)
        for i in range(C // 32):
            for j in range(F // 32):
                nc.vector.transpose(
                    expT_sb[32 * j:32 * j + 32, 32 * i:32 * i + 32],
                    exp_sb[32 * i:32 * i + 32, 32 * j:32 * j + 32])
        s_sb = const.tile([128, 1], f32)
        nc.vector.reduce_sum(s_sb[:], expT_sb[:], axis=AX.X)
        rec_sb = const.tile([128, 1], f32)
        nc.vector.reciprocal(rec_sb[:], s_sb[:])
        wT_sb = const.tile([128, C], f32)
        nc.vector.tensor_scalar_mul(wT_sb[:], expT_sb[:], rec_sb[:])
        w2_sb = const.tile([128, F], f32)
        for i in range(F // 32):
            for j in range(C // 32):
                nc.vector.transpose(
                    w2_sb[32 * j:32 * j + 32, 32 * i:32 * i + 32],
                    wT_sb[32 * i:32 * i + 32, 32 * j:32 * j + 32])
        nc.scalar.dma_start(w2_sb[C:128, :], w2_sb[0:C, :])

        # fp16 block-diagonal Wstack for the paired edge-score matmuls.
        wstack = const.tile([128, 2 * F], f16)
        nc.vector.memset(wstack[:], 0.0)
        nc.vector.tensor_copy(wstack[0:C, 0:F], w2_sb[0:C, :])
        nc.vector.tensor_copy(wstack[C:128, F:2 * F], w2_sb[C:128, :])

        out_sb = const.tile([128, NPC], f32)

        cnt_pool = ctx.enter_context(tc.tile_pool(name="cnt", bufs=3))
        idx_pool = ctx.enter_context(tc.tile_pool(name="idx", bufs=4))
        gath_pool = ctx.enter_context(tc.tile_pool(name="gath", bufs=5))
        es_pool = ctx.enter_context(tc.tile_pool(name="es", bufs=4, space="PSUM"))
        msgs_pool = ctx.enter_context(tc.tile_pool(name="msgs", bufs=6))
        ne_pool = ctx.enter_context(tc.tile_pool(name="ne", bufs=2, space="PSUM"))
        ns_pool = ctx.enter_context(tc.tile_pool(name="ns", bufs=1, space="PSUM"))
        red_pool = ctx.enter_context(tc.tile_pool(name="red", bufs=3))
        rns_pool = ctx.enter_context(tc.tile_pool(name="rns", bufs=3))

        gq = 0  # rotating SWDGE queue index

        # ---- main loop over 128-node windows ----
        # Edges are padded to whole windows with cnt=0 / idx=0, so every
        # window runs the full 32 tiles; only the final column copies are
        # restricted to the window's real node count.
        for w in range(n_win):
            nodes_w = min(WIN_NODES, NPC - w * WIN_NODES)
            nt = (nodes_w * DEG) // 128      # real edge tiles (32; last: 17)
            n_idx = nt * 128

            cnt_sb = cnt_pool.tile([128, HALF], f16, tag="cnt")
            nc.sync.dma_start(cnt_sb[:], cntp[:, w * HALF:(w + 1) * HALF])

            idx_sb = idx_pool.tile([128, EPW // 16], i16, tag="idx")
            nc.sync.dma_start(
                idx_sb[:, :n_idx // 16],
                idx[:, w * (EPW // 16): w * (EPW // 16) + n_idx // 16])

            gath = gath_pool.tile([128, EPW], f32, tag="gath")
            if "gather" in skip:
                nc.vector.memset(gath[:], 1.0)
            else:
                gath3 = gath[:].rearrange("p (t f) -> p t f", f=F)
                for e0 in range(0, n_idx, GATHER_CHUNK):
                    ecnt = min(GATHER_CHUNK, n_idx - e0)
                    nc.gpsimd.dma_gather(
                        out_ap=gath3[:, e0 // 128:(e0 + ecnt) // 128, :],
                        in_ap=emb[:, :],
                        idxs_ap=idx_sb[:, e0 // 16:(e0 + ecnt) // 16],
                        num_idxs=ecnt,
                        num_idxs_reg=ecnt,
                        elem_size=F,
                        queue_num=gq,
                    )
                    gq = (gq + 1) % N_QUEUES

            ne_ps = ne_pool.tile([128, 128], f32, tag="ne")
            g3 = gath[:].rearrange("p (t f) -> p t f", f=F)
            if nt == 32:
                # two tile pairs (j, j+16), (j+1, j+17) per PSUM bank; one
                # [128, 512] DVE multiply for all four tiles.
                for j in range(0, 16, 2):
                    es_ps = es_pool.tile([128, 512], f32, tag="es")
                    nc.tensor.matmul(
                        es_ps[:, 0:2 * F], cnt_sb[:, 128 * j:128 * (j + 1)],
                        wstack[:], start=True, stop=True)
                    nc.tensor.matmul(
                        es_ps[:, 2 * F:4 * F],
                        cnt_sb[:, 128 * (j + 1):128 * (j + 2)],
                        wstack[:], start=True, stop=True)
                    msgs = msgs_pool.tile([128, 512], f16, tag="msgs")
                    g4 = gath[:].rearrange(
                        "p (h j f) -> p j h f", h=2, f=F)  # t = h*16 + j
                    nc.vector.tensor_mul(
                        msgs[:].rearrange("p (j h f) -> p j h f", j=2, f=F),
                        g4[:, j:j + 2, :, :],
                        es_ps[:].rearrange("p (j h f) -> p j h f", j=2, f=F),
                    )
                    for a, t in enumerate((j, j + 16, j + 1, j + 17)):
                        nc.tensor.matmul(
                            ne_ps[:, 4 * t:4 * t + 4],
                            msgs[:, a * F:(a + 1) * F], pbase_sb[:],
                            start=True, stop=True,
                        )
            else:
                for j in range(min(nt, 16)):
                    has_hi = j + 16 < nt
                    es_ps = es_pool.tile([128, 512], f32, tag="es")
                    nw = 2 * F if has_hi else F
                    nc.tensor.matmul(
                        es_ps[:, :nw], cnt_sb[:, 128 * j:128 * (j + 1)],
                        wstack[:, :nw],
                        start=True, stop=True,
                    )
                    msgs = msgs_pool.tile([128, 512], f16, tag="msgs")
                    if has_hi:
                        nc.vector.tensor_mul(
                            msgs[:, :2 * F].rearrange("p (t f) -> p t f", f=F),
                            g3[:, j:j + 17:16, :],
                            es_ps[:, :2 * F].rearrange("p (t f) -> p t f", f=F),
                        )
                    else:
                        nc.vector.tensor_mul(
                            msgs[:, 0:F], g3[:, j, :], es_ps[:, 0:F])
                    nc.tensor.matmul(
                        ne_ps[:, 4 * j:4 * j + 4],
                        msgs[:, 0:F], pbase_sb[:],
                        start=True, stop=True,
                    )
                    if has_hi:
                        nc.tensor.matmul(
                            ne_ps[:, 64 + 4 * j:64 + 4 * j + 4],
                            msgs[:, F:2 * F], pbase_sb[:],
                            start=True, stop=True,
                        )

            # node_score path: segsum(cnt) on DVE, then ns^T = W^T @ segsum.
            # Two f32 matmuls in position-dedicated PSUM banks (lo: PE rows
            # 0-63, hi: rows 64-127) to honor the PE-row-position rule.
            # (NB: running this as a GpSimd add tree measured 6.5x SLOWER
            # end-to-end - Q7 elementwise ops stall the gather pipeline.)
            red_sb = red_pool.tile([128, 64], f32, tag="red")
            cnt3 = cnt_sb[:].rearrange("p (g d) -> p g d", d=DEG)
            nc.vector.reduce_sum(red_sb[:], cnt3, axis=AX.X)
            lo_w = min(64, nodes_w)
            hi_w = nodes_w - lo_w
            ns_lo = ns_pool.tile([128, 64], f32, tag="ns_lo")
            nc.tensor.matmul(ns_lo[:], w2_sb[0:64, :], red_sb[0:64, :],
                             start=True, stop=True)
            rns_sb = rns_pool.tile([128, 128], f32, tag="rns")
            nc.vector.reciprocal_approx_fast(rns_sb[:, :lo_w], ns_lo[:, :lo_w])
            if hi_w > 0:
                ns_hi = ns_pool.tile([128, 64], f32, tag="ns_hi")
                nc.tensor.matmul(ns_hi[:], w2_sb[64:128, :], red_sb[64:128, :],
                                 start=True, stop=True)
                nc.vector.reciprocal_approx_fast(rns_sb[:, 64:64 + hi_w], ns_hi[:, :hi_w])
            nc.vector.tensor_mul(
                out_sb[:, w * WIN_NODES:w * WIN_NODES + nodes_w],
                ne_ps[:, :nodes_w],
                rns_sb[:, :nodes_w],
            )
            # stream the output per window so the final DMA isn't one big
            # barrier at the end of the kernel
            nc.sync.dma_start(
                out[:, w * WIN_NODES:w * WIN_NODES + nodes_w],
                out_sb[:, w * WIN_NODES:w * WIN_NODES + nodes_w])

    nc.compile()
    return nc


def get_nc():
    if "nc" not in _CACHE:
        _CACHE["nc"] = _build_nc()
    return _CACHE["nc"]


def prep_in_maps(inputs):
    cnt = np.asarray(inputs["cnt"], dtype=np.float32)
    emb = np.ascontiguousarray(np.asarray(inputs["embedding"], dtype=np.float32))
    imp = np.ascontiguousarray(np.asarray(inputs["importance"], dtype=np.float32))
    src = np.asarray(inputs["src"], dtype=np.int64)
    dst = np.asarray(inputs["dst"], dtype=np.int64)

    perm = np.argsort(dst, kind="stable")
    src_s = src[perm]
    cnt_s = cnt[perm].astype(np.float16)

    pbase = np.zeros((128, 4), np.float16)
    pbase[np.arange(128), np.arange(128) // DEG] = 1.0

    in_maps = []
    for c in range(N_CORES):
        sl = slice(c * EPC, (c + 1) * EPC)
        cnt_core = np.zeros((PAD_EPC, C), np.float16)
        cnt_core[:EPC] = cnt_s[sl]
        src_core = np.zeros((PAD_EPC,), np.int64)
        src_core[:EPC] = src_s[sl]
        # half-pack: [w, half, j, c] -> [half*64+c, w*HALF+j]
        cc = cnt_core.reshape(N_WIN, 2, HALF, C)
        cntp = np.ascontiguousarray(
            cc.transpose(1, 3, 0, 2).reshape(128, N_WIN * HALF))
        # wrapped int16 index layout: idx i at [i%16, i//16], replicated x8
        idxw = np.ascontiguousarray(
            np.tile(src_core.reshape(PAD_EPC // 16, 16).T, (8, 1)).astype(np.int16))
        in_maps.append({
            "cntp": cntp,
            "idx": idxw,
            "emb": emb,
            "imp": imp,
            "pbase": pbase,
        })
    return in_maps


def unshard(core_outs):
    # each core out: [F, NPC] (F-major); concat over node axis, transpose.
    full = np.concatenate(core_outs, axis=1)          # [F, N]
    return np.ascontiguousarray(full.T.astype(np.float32))


def run(inputs, trace=False):
    from concourse.bass_utils import run_bass_kernel_spmd

    nc = get_nc()
    in_maps = prep_in_maps(inputs)
    res = run_bass_kernel_spmd(
        nc, in_maps, core_ids=list(range(N_CORES)), trace=trace)
    outs = [res.results[i]["out"] for i in range(N_CORES)]
    return unshard(outs), res


def kernel(**inputs):
    out, _ = run(inputs, trace=False)
    return out


# revision 42
# speedup vs baseline: 7.9096x; 1.0450x over previous
"""Trainium2 Bass kernel for nn_FOGCNConv (GNN message passing).

Math (reference):
    weight = softmax(importance, axis=0)            # [C, F]
    edge_score = cnt @ weight                       # [E, F]
    msgs = embedding[src] * edge_score              # [E, F]
    new_embedding = segment_sum(msgs, dst, N)       # [N, F]
    node_score = segment_sum(edge_score, dst, N)    # [N, F]
    out = new_embedding / node_score

Key structural facts (hardcoded; guaranteed by the input spec):
    N=20000 nodes, E=640000 edges, C=64, F=128, and dst is a permutation of
    arange(E) % N  =>  every node has exactly DEG=32 incoming edges.

Strategy:
  - Host: sort edges by dst. Then the two segment-sums become perfectly
    regular reductions over groups of 32 consecutive edges.
  - Shard *contiguous dst ranges* across the 8 cores (2500 nodes / 80000
    edges each) => fully independent cores, no collectives.
  - Device (per core, per 128-node window = 4096 edges = 32 edge tiles):
      * dma_gather embedding rows for the window's src indices
        (edge e -> partition e%128, free block e//128), 1024 idxs per
        gather (SWDGE ring capacity), rotated over 4 SWDGE queues.
      * edge_score for tile pair (j, j+16) in ONE K=128 fp16 matmul:
        lhsT = packed cnt^T [128, 128] (c-halves stacked), rhs = Wstack
        [128, 256] block-diagonal softmax weights -> es [128e, 256] f32.
      * msgs = gathered * edge_score on DVE (one 3D-strided op per pair),
        cast to fp16.
      * new_embedding window [128f, 128n] via PE "segment matmuls": per
        edge tile t, out[:, 4t:4t+4] = msgs_t^T @ P where P[e, j] =
        (e//32 == j) is a constant one-hot (edges are dst-sorted).
      * node_score factored: segsum(cnt) via DVE strided reduce ->
        block-diagonal fp16 redx, then ONE matmul ns^T = w2^T @ redx.
      * out window = new_embedding * 1/node_score (DVE), F-major.
  - cnt is fed pre-transposed fp16 and "half-packed" to [128, ...] so DMA
    uses all 128 partitions: partition (half*64 + c) holds cnt^T[c] for
    the window's half-th group of 2048 edges.
  - PE-row-position rule: a PSUM bank must only ever be written by matmuls
    whose stationary operand sits at one SBUF base partition. All matmuls
    here use base partition 0. (Mixing positions in one bank crashes the
    device; so does gpsimd.partition_all_reduce, and so does a dma_gather
    of more than ring-capacity indices.)
"""

import sys

if "/opt/trn_rl_repo" not in sys.path:
    sys.path.insert(0, "/opt/trn_rl_repo")

import numpy as np

# Problem sizes (fixed by the spec).
N_NODES = 20000
N_EDGES = 640000
C = 64
F = 128
N_CORES = 8
NPC = N_NODES // N_CORES       # 2500 nodes per core
EPC = N_EDGES // N_CORES       # 80000 edges per core
DEG = N_EDGES // N_NODES       # 32 edges per node
WIN_NODES = 128                # nodes per window
EPW = WIN_NODES * DEG          # 4096 edges per window
N_WIN = -(-NPC // WIN_NODES)   # 20 windows per core (last partial: 68 nodes)
PAD_EPC = N_WIN * EPW          # 81920 padded edges per core
HALF = EPW // 2                # 2048
GATHER_CHUNK = 1024            # max idxs per dma_gather (SWDGE ring capacity)
N_QUEUES = 4                   # SWDGE queues; rotate gathers across them

_CACHE = {}


def _build_nc(n_win=N_WIN, skip=()):
    import concourse.bass as bass  # noqa: F401
    import concourse.bacc as bacc
    import concourse.tile as tile
    import concourse.mybir as mybir
    from contextlib import ExitStack

    f32 = mybir.dt.float32
    f16 = mybir.dt.float16
    i16 = mybir.dt.int16
    AF = mybir.ActivationFunctionType
    AX = mybir.AxisListType

    nc = bacc.Bacc("TRN2", target_bir_lowering=False, debug=False,
                   num_swdge_queues=N_QUEUES)
    cntp = nc.declare_dram_parameter("cntp", [128, N_WIN * HALF], f16, isOutput=False)
    idx = nc.declare_dram_parameter("idx", [128, PAD_EPC // 16], i16, isOutput=False)
    emb = nc.declare_dram_parameter("emb", [N_NODES, F], f16, isOutput=False)
    imp = nc.declare_dram_parameter("imp", [C, F], f32, isOutput=False)
    pbase = nc.declare_dram_parameter("pbase", [128, 4], f16, isOutput=False)
    out = nc.declare_dram_parameter("out", [F, NPC], f32, isOutput=True)

    with ExitStack() as ctx:
        tc = ctx.enter_context(tile.TileContext(nc))
        const = ctx.enter_context(tc.tile_pool(name="const", bufs=1))

        # ---- constants ----
        pbase_sb = const.tile([128, 4], f16)
        nc.sync.dma_start(pbase_sb[:], pbase[:, :])

        # ---- softmax(importance, axis=0) on DVE (block transposes) ----
        imp_sb = const.tile([C, F], f32)
        nc.sync.dma_start(imp_sb[:], imp[:, :])
        exp_sb = const.tile([C, F], f32)
        nc.scalar.activation(exp_sb[:], imp_sb[:], AF.Exp)
        expT_sb = const.tile([128, C], f32)
        for i in range(C // 32):
            for j in range(F // 32):
                nc.vector.transpose(
                    expT_sb[32 * j:32 * j + 32, 32 * i:32 * i + 32],
                    exp_sb[32 * i:32 * i + 32, 32 * j:32 * j + 32])
        s_sb = const.tile([128, 1], f32)
        nc.vector.reduce_sum(s_sb[:], expT_sb[:], axis=AX.X)
        rec_sb = const.tile([128, 1], f32)
        nc.vector.reciprocal(rec_sb[:], s_sb[:])
        wT_sb = const.tile([128, C], f32)
        nc.vector.tensor_scalar_mul(wT_sb[:], expT_sb[:], rec_sb[:])
        w2_sb = const.tile([128, F], f32)
        for i in range(F // 32):
            for j in range(C // 32):
                nc.vector.transpose(
                    w2_sb[32 * j:32 * j + 32, 32 * i:32 * i + 32],
                    wT_sb[32 * i:32 * i + 32, 32 * j:32 * j + 32])
        nc.scalar.dma_start(w2_sb[C:128, :], w2_sb[0:C, :])

        # fp16 block-diagonal Wstack for the paired edge-score matmuls.
        wstack = const.tile([128, 2 * F], f16)
        nc.vector.memset(wstack[:], 0.0)
        nc.vector.tensor_copy(wstack[0:C, 0:F], w2_sb[0:C, :])
        nc.vector.tensor_copy(wstack[C:128, F:2 * F], w2_sb[C:128, :])

        out_sb = const.tile([128, NPC], f32)

        cnt_pool = ctx.enter_context(tc.tile_pool(name="cnt", bufs=3))
        idx_pool = ctx.enter_context(tc.tile_pool(name="idx", bufs=4))
        gath_pool = ctx.enter_context(tc.tile_pool(name="gath", bufs=5))
        es_pool = ctx.enter_context(tc.tile_pool(name="es", bufs=4, space="PSUM"))
        msgs_pool = ctx.enter_context(tc.tile_pool(name="msgs", bufs=6))
        ne_pool = ctx.enter_context(tc.tile_pool(name="ne", bufs=2, space="PSUM"))
        ns_pool = ctx.enter_context(tc.tile_pool(name="ns", bufs=1, space="PSUM"))
        red_pool = ctx.enter_context(tc.tile_pool(name="red", bufs=3))
        rns_pool = ctx.enter_context(tc.tile_pool(name="rns", bufs=3))

        gq = 0  # rotating SWDGE queue index

        # ---- main loop over 128-node windows ----
        # Edges are padded to whole windows with cnt=0 / idx=0, so every
        # window runs the full 32 tiles; only the final column copies are
        # restricted to the window's real node count.
        for w in range(n_win):
            nodes_w = min(WIN_NODES, NPC - w * WIN_NODES)
            nt = (nodes_w * DEG) // 128      # real edge tiles (32; last: 17)
            n_idx = nt * 128

            cnt_sb = cnt_pool.tile([128, HALF], f16, tag="cnt")
            nc.sync.dma_start(cnt_sb[:], cntp[:, w * HALF:(w + 1) * HALF])

            idx_sb = idx_pool.tile([128, EPW // 16], i16, tag="idx")
            nc.sync.dma_start(
                idx_sb[:, :n_idx // 16],
                idx[:, w * (EPW // 16): w * (EPW // 16) + n_idx // 16])

            gath = gath_pool.tile([128, EPW], f16, tag="gath")
            if "gather" in skip:
                nc.vector.memset(gath[:], 1.0)
            else:
                gath3 = gath[:].rearrange("p (t f) -> p t f", f=F)
                for e0 in range(0, n_idx, GATHER_CHUNK):
                    ecnt = min(GATHER_CHUNK, n_idx - e0)
                    nc.gpsimd.dma_gather(
                        out_ap=gath3[:, e0 // 128:(e0 + ecnt) // 128, :],
                        in_ap=emb[:, :],
                        idxs_ap=idx_sb[:, e0 // 16:(e0 + ecnt) // 16],
                        num_idxs=ecnt,
                        num_idxs_reg=ecnt,
                        elem_size=F,
                        queue_num=gq,
                    )
                    gq = (gq + 1) % N_QUEUES

            ne_ps = ne_pool.tile([128, 128], f32, tag="ne")
            g3 = gath[:].rearrange("p (t f) -> p t f", f=F)
            if nt == 32:
                # two tile pairs (j, j+16), (j+1, j+17) per PSUM bank; one
                # [128, 512] DVE multiply for all four tiles.
                for j in range(0, 16, 2):
                    es_ps = es_pool.tile([128, 512], f32, tag="es")
                    nc.tensor.matmul(
                        es_ps[:, 0:2 * F], cnt_sb[:, 128 * j:128 * (j + 1)],
                        wstack[:], start=True, stop=True)
                    nc.tensor.matmul(
                        es_ps[:, 2 * F:4 * F],
                        cnt_sb[:, 128 * (j + 1):128 * (j + 2)],
                        wstack[:], start=True, stop=True)
                    msgs = msgs_pool.tile([128, 512], f16, tag="msgs")
                    g4 = gath[:].rearrange(
                        "p (h j f) -> p j h f", h=2, f=F)  # t = h*16 + j
                    nc.vector.tensor_mul(
                        msgs[:].rearrange("p (j h f) -> p j h f", j=2, f=F),
                        g4[:, j:j + 2, :, :],
                        es_ps[:].rearrange("p (j h f) -> p j h f", j=2, f=F),
                    )
                    for a, t in enumerate((j, j + 16, j + 1, j + 17)):
                        nc.tensor.matmul(
                            ne_ps[:, 4 * t:4 * t + 4],
                            msgs[:, a * F:(a + 1) * F], pbase_sb[:],
                            start=True, stop=True,
                        )
            else:
                for j in range(min(nt, 16)):
                    has_hi = j + 16 < nt
                    es_ps = es_pool.tile([128, 512], f32, tag="es")
                    nw = 2 * F if has_hi else F
                    nc.tensor.matmul(
                        es_ps[:, :nw], cnt_sb[:, 128 * j:128 * (j + 1)],
                        wstack[:, :nw],
                        start=True, stop=True,
                    )
                    msgs = msgs_pool.tile([128, 512], f16, tag="msgs")
                    if has_hi:
                        nc.vector.tensor_mul(
                            msgs[:, :2 * F].rearrange("p (t f) -> p t f", f=F),
                            g3[:, j:j + 17:16, :],
                            es_ps[:, :2 * F].rearrange("p (t f) -> p t f", f=F),
                        )
                    else:
                        nc.vector.tensor_mul(
                            msgs[:, 0:F], g3[:, j, :], es_ps[:, 0:F])
                    nc.tensor.matmul(
                        ne_ps[:, 4 * j:4 * j + 4],
                        msgs[:, 0:F], pbase_sb[:],
                        start=True, stop=True,
                    )
                    if has_hi:
                        nc.tensor.matmul(
                            ne_ps[:, 64 + 4 * j:64 + 4 * j + 4],
                            msgs[:, F:2 * F], pbase_sb[:],
                            start=True, stop=True,
                        )

            # node_score path: segsum(cnt) on DVE, then ns^T = W^T @ segsum.
            # Two f32 matmuls in position-dedicated PSUM banks (lo: PE rows
            # 0-63, hi: rows 64-127) to honor the PE-row-position rule.
            # (NB: running this as a GpSimd add tree measured 6.5x SLOWER
            # end-to-end - Q7 elementwise ops stall the gather pipeline.)
            red_sb = red_pool.tile([128, 64], f32, tag="red")
            cnt3 = cnt_sb[:].rearrange("p (g d) -> p g d", d=DEG)
            nc.vector.reduce_sum(red_sb[:], cnt3, axis=AX.X)
            lo_w = min(64, nodes_w)
            hi_w = nodes_w - lo_w
            ns_lo = ns_pool.tile([128, 64], f32, tag="ns_lo")
            nc.tensor.matmul(ns_lo[:], w2_sb[0:64, :], red_sb[0:64, :],
                             start=True, stop=True)
            rns_sb = rns_pool.tile([128, 128], f32, tag="rns")
            nc.vector.reciprocal_approx_fast(rns_sb[:, :lo_w], ns_lo[:, :lo_w])
            if hi_w > 0:
                ns_hi = ns_pool.tile([128, 64], f32, tag="ns_hi")
                nc.tensor.matmul(ns_hi[:], w2_sb[64:128, :], red_sb[64:128, :],
                                 start=True, stop=True)
                nc.vector.reciprocal_approx_fast(rns_sb[:, 64:64 + hi_w], ns_hi[:, :hi_w])
            nc.vector.tensor_mul(
                out_sb[:, w * WIN_NODES:w * WIN_NODES + nodes_w],
                ne_ps[:, :nodes_w],
                rns_sb[:, :nodes_w],
            )
            # stream the output per window so the final DMA isn't one big
            # barrier at the end of the kernel
            nc.sync.dma_start(
                out[:, w * WIN_NODES:w * WIN_NODES + nodes_w],
                out_sb[:, w * WIN_NODES:w * WIN_NODES + nodes_w])

    nc.compile()
    return nc


def get_nc():
    if "nc" not in _CACHE:
        _CACHE["nc"] = _build_nc()
    return _CACHE["nc"]


def prep_in_maps(inputs):
    cnt = np.asarray(inputs["cnt"], dtype=np.float32)
    emb = np.ascontiguousarray(np.asarray(inputs["embedding"], dtype=np.float16))
    imp = np.ascontiguousarray(np.asarray(inputs["importance"], dtype=np.float32))
    src = np.asarray(inputs["src"], dtype=np.int64)
    dst = np.asarray(inputs["dst"], dtype=np.int64)

    perm = np.argsort(dst, kind="stable")
    src_s = src[perm]
    cnt_s = cnt[perm].astype(np.float16)

    pbase = np.zeros((128, 4), np.float16)
    pbase[np.arange(128), np.arange(128) // DEG] = 1.0

    in_maps = []
    for c in range(N_CORES):
        sl = slice(c * EPC, (c + 1) * EPC)
        cnt_core = np.zeros((PAD_EPC, C), np.float16)
        cnt_core[:EPC] = cnt_s[sl]
        src_core = np.zeros((PAD_EPC,), np.int64)
        src_core[:EPC] = src_s[sl]
        # half-pack: [w, half, j, c] -> [half*64+c, w*HALF+j]
        cc = cnt_core.reshape(N_WIN, 2, HALF, C)
        cntp = np.ascontiguousarray(
            cc.transpose(1, 3, 0, 2).reshape(128, N_WIN * HALF))
        # wrapped int16 index layout: idx i at [i%16, i//16], replicated x8
        idxw = np.ascontiguousarray(
            np.tile(src_core.reshape(PAD_EPC // 16, 16).T, (8, 1)).astype(np.int16))
        in_maps.append({
            "cntp": cntp,
            "idx": idxw,
            "emb": emb,
            "imp": imp,
            "pbase": pbase,
        })
    return in_maps


def unshard(core_outs):
    # each core out: [F, NPC] (F-major); concat over node axis, transpose.
    full = np.concatenate(core_outs, axis=1)          # [F, N]
    return np.ascontiguousarray(full.T.astype(np.float32))


def run(inputs, trace=False):
    from concourse.bass_utils import run_bass_kernel_spmd

    nc = get_nc()
    in_maps = prep_in_maps(inputs)
    res = run_bass_kernel_spmd(
        nc, in_maps, core_ids=list(range(N_CORES)), trace=trace)
    outs = [res.results[i]["out"] for i in range(N_CORES)]
    return unshard(outs), res


def kernel(**inputs):
    out, _ = run(inputs, trace=False)
    return out
